# revision 1
# baseline (speedup 1.0000x reference)
"""Sliding-window (tau=32) multi-head attention block with shared qkv projection,
distributed over 8 trn2 NeuronCores.

Sharding: data/sequence-parallel over the flattened (batch, token) axis —
8 shards of 1024 tokens. Each core receives its k/v slice with a 32-row
front halo (zeros at batch start), so projecting the concatenated buffer
reproduces the reference's pad-then-project semantics exactly (incl. bias).

Speed notes (cost-model driven):
- matmul moving operand bf16 => 1 cycle/row at any width (f32r <256-wide
  runs at 1/4 rate at full clock). qpT/kpT/vpa/aw are bf16.
- transposes stream an f32r identity (1.5 cyc/row; bf16 identity is
  rejected by the walrus verifier when paired with f32r data).
- parity-slot score layout: q-block g always lands at column slot g%2, so
  the out matmul for even g contracts 64 kv rows in ONE matmul; odd g
  splits into po/po2 banks, combined by one dual-PSUM tensor op.
- GPSIMD (Pool) cannot access PSUM: it gets only SBUF work (mask slot0
  multiply + final normalize multiply).
- vpa projected with plain wT (256-wide); ones columns persist in SBUF.
- batched DMA loads/stores (565ns SP issue cost per dma_start).
"""

import numpy as np

import concourse.bacc as bacc
import concourse.bass as bass
import concourse.tile as tile
from concourse import mybir
from concourse.bass_utils import run_bass_kernel_spmd

B, N, E = 2, 4096, 256
H, TAU = 8, 32
HD = E // H
SCALING = HD**-0.5

NCORES = 8
T = B * N // NCORES  # 1024 q tokens per core
KT = T + 32  # kv rows incl. 32-row front halo
NB = KT // 32  # 33 kv blocks of 32
NKTILE = 9  # score tiles of (up to) 4 kv blocks

F32 = mybir.dt.float32
F32R = mybir.dt.float32r
BF16 = mybir.dt.bfloat16


def _host_constants():
    """Dense band mask [128, 160]: chunk-local kv row j, window col i
    (q = 128c - 32 + i, kv_s = 128c + j); valid iff i - j in [0, 31]."""
    j = np.arange(128)[:, None]
    i = np.arange(160)[None, :]
    return ((i - j >= 0) & (i - j <= 31)).astype(np.float32)


def _prep_weights(W, b):
    WT = np.ascontiguousarray(W.T).astype(np.float32)  # [e_in, e_out]
    wT = WT.reshape(2, 128, 256).copy()
    b2 = b.reshape(2, 128).astype(np.float32)
    b2s = (SCALING * b2).astype(np.float32)
    brow = b.reshape(1, 256).astype(np.float32)
    return wT, b2, b2s, brow


def build_program(stage=4, reps=1, bias_zero=True):
    # stage: 1=DMA only, 2=+transposes, 3=+q/k proj, 3.5=+vpa, 3.7=+scores,
    # 4/full pipeline. Debug stages dump a slice to out_d to defeat DCE.
    nc = bacc.Bacc("TRN2", target_bir_lowering=False)

    q_d = nc.dram_tensor("q", [T, E], F32, kind="ExternalInput")
    k_d = nc.dram_tensor("k", [KT, E], F32, kind="ExternalInput")
    v_d = nc.dram_tensor("v", [KT, E], F32, kind="ExternalInput")
    wT_d = nc.dram_tensor("wT", [2, 128, 256], F32, kind="ExternalInput")
    brow_d = nc.dram_tensor("brow", [1, 256], F32, kind="ExternalInput")
    b2_d = nc.dram_tensor("b2", [2, 128], F32, kind="ExternalInput")
    b2s_d = nc.dram_tensor("b2s", [2, 128], F32, kind="ExternalInput")
    ident_d = nc.dram_tensor("ident", [128, 128], F32, kind="ExternalInput")
    ones_d = nc.dram_tensor("ones_row", [1, 128], F32, kind="ExternalInput")
    mask_d = nc.dram_tensor("mask", [128, 160], F32, kind="ExternalInput")
    out_d = nc.dram_tensor("out", [T, E], F32, kind="ExternalOutput")

    with tile.TileContext(nc) as tc:
        with (
            tc.tile_pool(name="consts", bufs=1) as consts,
            tc.tile_pool(name="raw", bufs=2) as raw_pool,
            tc.tile_pool(name="xT", bufs=1) as xT_pool,
            tc.tile_pool(name="proj", bufs=1) as proj_pool,
            tc.tile_pool(name="vpa", bufs=1) as vpa_pool,
            tc.tile_pool(name="aw", bufs=1) as aw_pool,
            tc.tile_pool(name="ofin", bufs=2) as ofin_pool,
            tc.tile_pool(name="ps_proj", bufs=2, space="PSUM") as ps_proj,
            tc.tile_pool(name="ps_s", bufs=1, space="PSUM") as ps_s,
            tc.tile_pool(name="ps_o", bufs=1, space="PSUM") as ps_o,
        ):
            # ---- constants -------------------------------------------------
            ident_f = consts.tile([128, 128], F32)
            nc.sync.dma_start(out=ident_f, in_=ident_d.ap())
            ident = consts.tile([128, 128], F32R)
            nc.vector.tensor_copy(ident, ident_f)
            ones_sb = consts.tile([1, 128], F32)
            nc.sync.dma_start(out=ones_sb, in_=ones_d.ap())
            ones_fr = consts.tile([1, 128], F32R)
            nc.vector.tensor_copy(ones_fr, ones_sb)
            brow_sb = consts.tile([1, 256], F32)
            nc.sync.dma_start(out=brow_sb, in_=brow_d.ap())
            brow_fr = consts.tile([1, 256], F32R)
            nc.vector.tensor_copy(brow_fr, brow_sb)
            b2_sb = consts.tile([128, 2], F32)
            b2s_sb = consts.tile([128, 2], F32)
            for o in range(2):
                nc.sync.dma_start(out=b2_sb[:, o : o + 1], in_=b2_d.ap()[o][:, None])
                nc.sync.dma_start(out=b2s_sb[:, o : o + 1], in_=b2s_d.ap()[o][:, None])
            wT_sb = consts.tile([128, 2, 256], F32)
            for ki in range(2):
                nc.sync.dma_start(out=wT_sb[:, ki, :], in_=wT_d.ap()[ki])
            wT_fr = consts.tile([128, 2, 256], F32R)
            nc.vector.tensor_copy(wT_fr, wT_sb)
            mask_f = consts.tile([128, 160], F32)
            nc.sync.dma_start(out=mask_f, in_=mask_d.ap())
            mask_sb = consts.tile([128, 160], BF16)
            nc.vector.tensor_copy(mask_sb, mask_f)

            # persistent vpa tiles: [128, H, 33] bf16, ones column at
            # [:, h, 32] initialized ONCE (drains only write [:, h, 0:32]).
            vpa = []
            for i in range(9):
                pc = 128 if i < 8 else 32
                t_ = vpa_pool.tile([pc, H, 33], BF16, tag=f"vpa{i}", name=f"vpa{i}")
                nc.vector.memset(t_[:, :, 32:33], 1.0)
                vpa.append(t_)

            def mask_bcast(c0, nc_):
                return bass.AP(
                    tensor=mask_sb.tensor,
                    offset=mask_sb.offset + c0,
                    ap=[mask_sb.ap[0], [0, 4], [0, 2], [1, nc_]],
                )

            # ---- per-rep body ---------------------------------------------
            for _rep in range(reps):
                raw_q = raw_pool.tile([128, 8, 256], F32R, tag="rq")
                raw_k = raw_pool.tile([128, 8, 256], F32R, tag="rk")
                raw_v = raw_pool.tile([128, 8, 256], F32R, tag="rv")
                raw_kt = raw_pool.tile([32, 256], F32R, tag="rkt")
                raw_vt = raw_pool.tile([32, 256], F32R, tag="rvt")
                # contiguous loads: partition j = tokens 8j..8j+8 (128 x 8KB
                # descriptors instead of 1024 x 1KB); the transpose drains
                # below re-stride so xT ends up in natural token order.
                nc.sync.dma_start(
                    out=raw_q.rearrange("p t e -> p (t e)"),
                    in_=q_d.ap().rearrange("(p x) e -> p (x e)", p=128).bitcast(F32R),
                )
                nc.sync.dma_start(
                    out=raw_k.rearrange("p t e -> p (t e)"),
                    in_=k_d.ap()[0:1024]
                    .rearrange("(p x) e -> p (x e)", p=128)
                    .bitcast(F32R),
                )
                nc.sync.dma_start(out=raw_kt, in_=k_d.ap()[1024:1056].bitcast(F32R))
                nc.sync.dma_start(
                    out=raw_v.rearrange("p t e -> p (t e)"),
                    in_=v_d.ap()[0:1024]
                    .rearrange("(p x) e -> p (x e)", p=128)
                    .bitcast(F32R),
                )
                nc.sync.dma_start(out=raw_vt, in_=v_d.ap()[1024:1056].bitcast(F32R))

                def debug_dump(sb_ap, width):
                    dst = out_d.ap()[0:128]
                    nc.sync.dma_start(out=dst[:, :width], in_=sb_ap)

                if stage < 2:
                    debug_dump(raw_q[:, 0, :].bitcast(F32), 256)
                    continue

                xT_q = xT_pool.tile([128, 2, T], F32R, tag="xTq")
                xT_k = xT_pool.tile([128, 2, KT], F32R, tag="xTk")
                xT_v = xT_pool.tile([128, 2, KT], F32R, tag="xTv")

                drain_alt = [0]

                def load_transpose(raw, tail, xT):
                    # raw[:, tt, :]: tokens {8j + tt}; transpose pairs of
                    # tt-slices, drain into xT cols {8j + tt} (stride 8).
                    xTv = xT.rearrange("p o (j t) -> p o j t", t=8)
                    for half in range(4):
                        tt = 2 * half
                        pt = ps_proj.tile(
                            [128, 512], F32, tag="psp", name="pt"
                        ).rearrange("p (a b) -> p a b", a=2)
                        for u in range(2):
                            for o in range(2):
                                nc.tensor.transpose(
                                    pt[:, o, 128 * u : 128 * u + 128].bitcast(F32R),
                                    raw[:, tt + u, 128 * o : 128 * o + 128],
                                    ident,
                                )
                        dst = xTv[:, :, 0:128, tt : tt + 2]
                        src = pt.rearrange("p a (u j) -> p a j u", u=2)
                        if drain_alt[0] % 2 == 0:
                            nc.scalar.activation(
                                dst, src, mybir.ActivationFunctionType.Copy
                            )
                        else:
                            nc.vector.tensor_copy(dst, src)
                        drain_alt[0] += 1
                    if tail is not None:
                        pt = ps_proj.tile(
                            [128, 512], F32, tag="psp", name="pt"
                        ).rearrange("p (a b) -> p a b", a=2)
                        for o in range(2):
                            nc.tensor.transpose(
                                pt[:, o, 0:32].bitcast(F32R),
                                tail[:, 128 * o : 128 * o + 128],
                                ident[:32, :32],
                            )
                        if drain_alt[0] % 2 == 0:
                            nc.scalar.activation(
                                xT[:, :, 1024:1056],
                                pt[:, :, 0:32],
                                mybir.ActivationFunctionType.Copy,
                            )
                        else:
                            nc.vector.tensor_copy(
                                xT[:, :, 1024:1056], pt[:, :, 0:32]
                            )
                        drain_alt[0] += 1

                load_transpose(raw_q, None, xT_q)
                load_transpose(raw_k, raw_kt, xT_k)
                load_transpose(raw_v, raw_vt, xT_v)

                if stage < 3:
                    debug_dump(xT_q[:, 0, 0:256].bitcast(F32), 256)
                    continue

                # ---- q/k projections -> bf16 transposed layout -------------
                qpT = proj_pool.tile([128, 2, T], BF16, tag="qpT", bufs=2)
                kpT = proj_pool.tile([128, 2, KT], BF16, tag="kpT", bufs=2)

                def project_T(xT, outT, tok_total, bias_sb, scale):
                    j = 0
                    while j < tok_total:
                        w = min(512, tok_total - j)
                        for o in range(2):
                            ps = ps_proj.tile([128, 512], F32, tag="psp")
                            for ki in range(2):
                                nc.tensor.matmul(
                                    ps[:, :w],
                                    wT_fr[:, ki, 128 * o : 128 * o + 128],
                                    xT[:, ki, j : j + w],
                                    start=(ki == 0),
                                    stop=(ki == 1),
                                )
                            if drain_alt[0] % 2 == 0:
                                nc.scalar.activation(
                                    outT[:, o, j : j + w],
                                    ps[:, :w],
                                    mybir.ActivationFunctionType.Identity,
                                    bias=bias_sb[:, o : o + 1],
                                    scale=scale,
                                )
                            else:
                                nc.vector.tensor_scalar(
                                    outT[:, o, j : j + w],
                                    ps[:, :w],
                                    scale,
                                    bias_sb[:, o : o + 1],
                                    mybir.AluOpType.mult,
                                    mybir.AluOpType.add,
                                )
                            drain_alt[0] += 1
                        j += w

                project_T(xT_q, qpT, T, b2_sb, 1.0)
                project_T(xT_k, kpT, KT, b2s_sb, SCALING)

                if stage < 3.5:
                    debug_dump(qpT[:, 0, 0:256].bitcast(F32), 128)
                    continue

                # ---- v projection (natural layout, plain wT) ---------------
                for i in range(9):
                    pc = 128 if i < 8 else 32
                    c0 = 128 * i
                    ps = ps_proj.tile([128, 512], F32, tag="psp")
                    for ki in range(2):
                        nc.tensor.matmul(
                            ps[:pc, 0:256],
                            xT_v[:, ki, c0 : c0 + pc],
                            wT_fr[:, ki, :],
                            start=(ki == 0),
                            stop=(bias_zero and ki == 1),
                        )
                    if not bias_zero:
                        nc.tensor.matmul(
                            ps[:pc, 0:256],
                            ones_fr[:, :pc],
                            brow_fr,
                            start=False,
                            stop=True,
                        )
                    src = ps[:pc, 0:256].rearrange("p (h x) -> p h x", h=H)
                    dst = vpa[i][:pc, :, 0:32]
                    if i % 2 == 0:
                        nc.vector.tensor_copy(dst, src)
                    else:
                        nc.scalar.activation(
                            dst, src, mybir.ActivationFunctionType.Copy
                        )

                if stage < 3.7:
                    nc.sync.dma_start(
                        out=out_d.ap()[0:128].bitcast(BF16)[:, 0:32],
                        in_=vpa[0][:, 0, 0:32],
                    )
                    continue

                # ---- dense masked scores + exp + mask + out ----------------
                # aw[c][j, hr, ht, i]: masked exp'd S^T for kv chunk c over
                # its 160-wide q window; invalid (i, j) pairs are 0, so the
                # out matmul contracts the full 128-row chunk in one go.
                aw = [
                    aw_pool.tile(
                        [128, 4, 2, 160], BF16, tag=f"aw{c}", name=f"aw{c}"
                    )
                    for c in range(NKTILE)
                ]
                obuf = ofin_pool.tile([128, 8, 256], F32, tag="obuf")

                def out_tile(t):
                    po = ps_o.tile(
                        [128, H, 33], F32, tag="po", name="po",
                        padded_shape=[128, H, 64], bufs=2,
                    )
                    nh = 4 if stage == 4.3 else H
                    for h in range(nh):
                        hr, ht = h % 4, h // 4
                        nc.tensor.matmul(
                            po[:, h, 0:33],
                            aw[t][:, hr, ht, 32:160],
                            vpa[t][:, h, :],
                            start=True,
                            stop=(stage == 4.2),
                            tile_position=(0, 0),
                            skip_group_check=True,
                        )
                        if stage != 4.2:
                            nc.tensor.matmul(
                                po[96:128, h, 0:33],
                                aw[t + 1][0:32, hr, ht, 0:32],
                                vpa[t + 1][0:32, h, :],
                                start=False,
                                stop=True,
                                tile_position=(0, 96),
                                skip_group_check=True,
                            )
                    if stage == 4.1:
                        nc.scalar.activation(
                            obuf[:, t, :].rearrange("p (h x) -> p h x", h=H),
                            po[:, :, 0:32],
                            mybir.ActivationFunctionType.Copy,
                        )
                    else:
                        rec = ofin_pool.tile([128, H], F32, tag="rec")
                        nc.vector.reciprocal(rec, po[:, :, 32])
                        rec_b = bass.AP(
                            tensor=rec.tensor,
                            offset=rec.offset,
                            ap=[rec.ap[0], [rec.ap[1][0], H], [0, 32]],
                        )
                        nc.vector.tensor_mul(
                            obuf[:, t, :].rearrange("p (h x) -> p h x", h=H),
                            po[:, :, 0:32],
                            rec_b,
                        )
                    if t == 3 or t == 7:
                        t0 = t - 3
                        nc.sync.dma_start(
                            out=out_d.ap()
                            .rearrange("(c p) e -> p c e", p=128)[:, t0 : t + 1, :],
                            in_=obuf[:, t0 : t + 1, :],
                        )

                for c in range(NKTILE):
                    ps = ps_s.tile(
                        [128, 4, 2, 160], F32, tag="pss",
                        padded_shape=[128, 4, 2, 256],
                    )
                    if c == 0:
                        i0, i1 = 32, 160  # window cols < 32 are q < 0
                        q0, q1 = 0, 128
                    elif c == NKTILE - 1:
                        i0, i1 = 0, 32  # only q blocks up to 1024 exist
                        q0, q1 = 992, 1024
                    else:
                        i0, i1 = 0, 160
                        q0, q1 = 128 * c - 32, 128 * c + 128
                    kw = 128 if c < NKTILE - 1 else 32
                    for h in range(H):
                        hr, ht = h % 4, h // 4
                        nc.tensor.matmul(
                            ps[0:kw, hr, ht, i0:i1],
                            kpT[
                                32 * hr : 32 * hr + 32, ht,
                                128 * c : 128 * c + kw,
                            ],
                            qpT[32 * hr : 32 * hr + 32, ht, q0:q1],
                            start=True,
                            stop=True,
                            tile_position=(32 * hr, 0),
                        )
                    ex = aw_pool.tile(
                        [128, 4, 2, 160], BF16, tag="ex", bufs=3
                    )
                    pp = 128 if c < NKTILE - 1 else 32
                    nc.scalar.activation(
                        ex[:pp, :, :, i0:i1],
                        ps[:pp, :, :, i0:i1],
                        mybir.ActivationFunctionType.Exp,
                    )
                    # first-32-col slice first (unblocks out_tile(c-1)'s
                    # halo matmuls), then the rest
                    if i0 == 0:
                        nc.gpsimd.tensor_mul(
                            aw[c][:pp, :, :, 0:32],
                            ex[:pp, :, :, 0:32],
                            bass.AP(
                                tensor=mask_sb.tensor,
                                offset=mask_sb.offset,
                                ap=[[mask_sb.ap[0][0], pp], [0, 4], [0, 2], [1, 32]],
                            ),
                        )
                        if stage >= 4 and c >= 1:
                            out_tile(c - 1)
                        if i1 > 32:
                            nc.vector.tensor_mul(
                                aw[c][:pp, :, :, 32:128],
                                ex[:pp, :, :, 32:128],
                                mask_bcast(32, 96),
                            )
                            nc.gpsimd.tensor_mul(
                                aw[c][:pp, :, :, 128:160],
                                ex[:pp, :, :, 128:160],
                                mask_bcast(128, 32),
                            )
                    else:
                        nc.vector.tensor_mul(
                            aw[c][:pp, :, :, i0:128],
                            ex[:pp, :, :, i0:128],
                            mask_bcast(i0, 128 - i0),
                        )
                        nc.gpsimd.tensor_mul(
                            aw[c][:pp, :, :, 128:i1],
                            ex[:pp, :, :, 128:i1],
                            mask_bcast(128, i1 - 128),
                        )
                        if stage >= 4 and c >= 1:
                            out_tile(c - 1)

                if stage < 4:
                    nc.sync.dma_start(
                        out=out_d.ap()[0:128].bitcast(BF16)[:, 0:128],
                        in_=aw[0][:, 0, 0, 32:160],
                    )

    nc.compile()
    return nc


_NC_CACHE = {}


def _get_nc(bias_zero):
    if bias_zero not in _NC_CACHE:
        _NC_CACHE[bias_zero] = build_program(bias_zero=bias_zero)
    return _NC_CACHE[bias_zero]


def make_in_maps(query, key, value, W, b):
    query = np.asarray(query, np.float32)
    key = np.asarray(key, np.float32)
    value = np.asarray(value, np.float32)
    W = np.asarray(W, np.float32)
    b = np.asarray(b, np.float32)

    wT, b2, b2s, brow = _prep_weights(W, b)
    mask = _host_constants()
    ident = np.eye(128, dtype=np.float32)
    ones_row = np.ones((1, 128), np.float32)

    qf = query.reshape(B * N, E)
    kf = key.reshape(B * N, E)
    vf = value.reshape(B * N, E)
    shards_per_b = NCORES // B
    in_maps = []
    for c in range(NCORES):
        s0 = c * T
        halo0 = s0 - 32
        if c % shards_per_b == 0:
            halo_k = np.zeros((32, E), np.float32)
            halo_v = np.zeros((32, E), np.float32)
        else:
            halo_k = kf[halo0:s0]
            halo_v = vf[halo0:s0]
        in_maps.append(
            {
                "q": np.ascontiguousarray(qf[s0 : s0 + T]),
                "k": np.ascontiguousarray(np.concatenate([halo_k, kf[s0 : s0 + T]])),
                "v": np.ascontiguousarray(np.concatenate([halo_v, vf[s0 : s0 + T]])),
                "wT": wT,
                "brow": brow,
                "b2": b2,
                "b2s": b2s,
                "ident": ident,
                "ones_row": ones_row,
                "mask": mask,
            }
        )
    return in_maps


def kernel(query, key, value, W, b):
    nc = _get_nc(not np.any(np.asarray(b)))
    in_maps = make_in_maps(query, key, value, W, b)
    res = run_bass_kernel_spmd(nc, in_maps, list(range(NCORES)))
    out = np.concatenate([res.results[c]["out"] for c in range(NCORES)], axis=0)
    return out.reshape(B, N, E).astype(np.float32)



# revision 2
# speedup vs baseline: 1.8999x; 1.8999x over previous
"""Sliding-window (tau=32) multi-head attention block with shared qkv
projection, distributed over 8 trn2 NeuronCores.  bf16 end-to-end with
DMA-crossbar transposed loads.

Sharding: sequence-parallel over the flattened (batch, token) axis — 8
shards of 1024 tokens; k/v carry a 32-row front halo (zeros at batch
starts) so projecting the concatenated buffer reproduces the reference's
pad-then-project semantics exactly (incl. bias).

Speed notes (cost-model driven):
- host converts q/k/v/W to bf16 and packs [q; k_halo; v_halo] into ONE
  [3136, 256] dram tensor; a single dma_start_transpose (xbar, 16x128
  tiles @14ns) lands x^T [128, 2, 3136] directly in SBUF: no PE
  transposes, no transpose drains, 1 load + 1 store DMA per rep.
- output is written bf16 [1024, 256]; host upcasts to f32.
- matmul cost = out_free_size x cycles/row(moving dtype); all matmuls
  keep bf16 moving operands.
- masked exp'd scores (aw) layout: dense [128kv, 160qwin] per chunk;
  parity-slot out matmuls contract whole chunks.
- Pool (GPSIMD) cannot touch PSUM: it gets SBUF mask multiplies only;
  Act keeps exp (act-table engine) plus a slice of drains; DVE takes
  packed bf16 copies (2x_1p), psum drains, and normalize.
"""

import numpy as np
import ml_dtypes

import concourse.bacc as bacc
import concourse.bass as bass
import concourse.tile as tile
from concourse import mybir
from concourse.bass_utils import run_bass_kernel_spmd

B, N, E = 2, 4096, 256
H, TAU = 8, 32
HD = E // H
SCALING = HD**-0.5

NCORES = 8
T = B * N // NCORES  # 1024 q tokens per core
KT = T + 32  # kv rows incl. 32-row front halo
NKTILE = 9  # kv chunks of (up to) 128
XN = T + 2 * KT  # 3136 rows in the packed transposed-load input

F32 = mybir.dt.float32
BF16 = mybir.dt.bfloat16
NPBF16 = ml_dtypes.bfloat16

X_Q = 0
X_K = T  # 1024
X_V = T + KT  # 2080


def _host_mask():
    """Dense band mask [128, 160]: chunk-local kv row j, window col i
    (q = 128c - 32 + i, kv_row = 128c + j); valid iff i - j in [0, 31]."""
    j = np.arange(128)[:, None]
    i = np.arange(160)[None, :]
    return ((i - j >= 0) & (i - j <= 31)).astype(NPBF16)


def build_program(stage=4, reps=1, bias_zero=True):
    nc = bacc.Bacc("TRN2", target_bir_lowering=False)

    xin_d = nc.dram_tensor("xin", [XN, E], BF16, kind="ExternalInput")
    wT_d = nc.dram_tensor("wT", [2, 128, 256], BF16, kind="ExternalInput")
    b2_d = nc.dram_tensor("b2", [2, 128], F32, kind="ExternalInput")
    b2s_d = nc.dram_tensor("b2s", [2, 128], F32, kind="ExternalInput")
    brow_d = nc.dram_tensor("brow", [1, 256], BF16, kind="ExternalInput")
    ones_d = nc.dram_tensor("ones_row", [1, 128], BF16, kind="ExternalInput")
    mask_d = nc.dram_tensor("mask", [128, 160], BF16, kind="ExternalInput")
    out_d = nc.dram_tensor("out", [T, E], BF16, kind="ExternalOutput")

    with tile.TileContext(nc) as tc:
        with (
            tc.tile_pool(name="consts", bufs=1) as consts,
            tc.tile_pool(name="xT", bufs=2) as xT_pool,
            tc.tile_pool(name="proj", bufs=1) as proj_pool,
            tc.tile_pool(name="vpa", bufs=1) as vpa_pool,
            tc.tile_pool(name="aw", bufs=1) as aw_pool,
            tc.tile_pool(name="ofin", bufs=2) as ofin_pool,
            tc.tile_pool(name="ps_proj", bufs=2, space="PSUM") as ps_proj,
            tc.tile_pool(name="ps_s", bufs=1, space="PSUM") as ps_s,
            tc.tile_pool(name="ps_o", bufs=1, space="PSUM") as ps_o,
        ):
            # ---- constants -------------------------------------------------
            ones_sb = consts.tile([1, 128], BF16)
            nc.sync.dma_start(out=ones_sb, in_=ones_d.ap())
            brow_sb = consts.tile([1, 256], BF16)
            nc.sync.dma_start(out=brow_sb, in_=brow_d.ap())
            b2_sb = consts.tile([128, 2], F32)
            b2s_sb = consts.tile([128, 2], F32)
            for o in range(2):
                nc.sync.dma_start(out=b2_sb[:, o : o + 1], in_=b2_d.ap()[o][:, None])
                nc.sync.dma_start(out=b2s_sb[:, o : o + 1], in_=b2s_d.ap()[o][:, None])
            wT_sb = consts.tile([128, 2, 256], BF16)
            for ki in range(2):
                nc.sync.dma_start(out=wT_sb[:, ki, :], in_=wT_d.ap()[ki])
            mask_sb = consts.tile([128, 160], BF16)
            nc.sync.dma_start(out=mask_sb, in_=mask_d.ap())

            # persistent vpa tiles: [128, H, 33] bf16, ones column at
            # [:, h, 32] initialized ONCE (drains only write [:, h, 0:32]).
            vpa = []
            for i in range(9):
                pc = 128 if i < 8 else 32
                t_ = vpa_pool.tile([pc, H, 33], BF16, tag=f"vpa{i}", name=f"vpa{i}")
                nc.vector.memset(t_[:, :, 32:33], 1.0)
                vpa.append(t_)

            def mask_bcast(c0, nc_):
                return bass.AP(
                    tensor=mask_sb.tensor,
                    offset=mask_sb.offset + c0,
                    ap=[mask_sb.ap[0], [0, 4], [0, 2], [1, nc_]],
                )

            # ---- per-rep body ---------------------------------------------
            xT_next = []
            for _rep in range(reps):
                # xbar-transposed load: xT[p, o, t] = xin[t, 128o + p];
                # reps 1+ use the tile prefetched mid-previous-rep
                if xT_next:
                    xT = xT_next.pop()
                else:
                    xT = xT_pool.tile([128, 2, XN], BF16, tag="xT")
                    nc.sync.dma_start_transpose(xT, xin_d.ap())
                xT_q = xT[:, :, X_Q : X_Q + T]
                xT_k = xT[:, :, X_K : X_K + KT]
                xT_v = xT[:, :, X_V : X_V + KT]

                def debug_dump(sb_ap, width):
                    dst = out_d.ap()[0:128]
                    nc.sync.dma_start(out=dst[:, :width], in_=sb_ap)

                if stage < 3:
                    debug_dump(xT[:, 0, 0:256], 256)
                    continue

                drain_alt = [0]

                # ---- q/k projections -> bf16 transposed layout -------------
                qpT = proj_pool.tile([128, 2, T], BF16, tag="qpT", bufs=2)
                kpT = proj_pool.tile([128, 2, KT], BF16, tag="kpT", bufs=2)

                def proj_chunk(xTx, outT, j, w, bias_sb, scale, on_act):
                    for o in range(2):
                        ps = ps_proj.tile([128, 512], F32, tag="psp")
                        for ki in range(2):
                            nc.tensor.matmul(
                                ps[:, :w],
                                wT_sb[:, ki, 128 * o : 128 * o + 128],
                                xTx[:, ki, j : j + w],
                                start=(ki == 0),
                                stop=(ki == 1),
                            )
                        if on_act and o == 1:
                            nc.scalar.activation(
                                outT[:, o, j : j + w],
                                ps[:, :w],
                                mybir.ActivationFunctionType.Identity,
                                bias=bias_sb[:, o : o + 1],
                                scale=scale,
                            )
                        else:
                            nc.vector.tensor_scalar(
                                outT[:, o, j : j + w],
                                ps[:, :w],
                                scale,
                                bias_sb[:, o : o + 1],
                                mybir.AluOpType.mult,
                                mybir.AluOpType.add,
                            )

                def vproj_chunk(i):
                    pc = 128 if i < 8 else 32
                    c0 = 128 * i
                    ps = ps_proj.tile([128, 512], F32, tag="psp")
                    for ki in range(2):
                        nc.tensor.matmul(
                            ps[:pc, 0:256],
                            xT_v[:, ki, c0 : c0 + pc],
                            wT_sb[:, ki, :],
                            start=(ki == 0),
                            stop=(bias_zero and ki == 1),
                        )
                    if not bias_zero:
                        nc.tensor.matmul(
                            ps[:pc, 0:256],
                            ones_sb[:, :pc],
                            brow_sb,
                            start=False,
                            stop=True,
                        )
                    nc.vector.tensor_copy(
                        vpa[i][:pc, :, 0:32],
                        ps[:pc, 0:256].rearrange("p (h x) -> p h x", h=H),
                    )

                if stage < 3.7:
                    # simple staged debug: full projections, then dump
                    proj_chunk(xT_q, qpT, 0, 512, b2_sb, 1.0, False)
                    proj_chunk(xT_q, qpT, 512, 512, b2_sb, 1.0, False)
                    proj_chunk(xT_k, kpT, 0, 512, b2s_sb, SCALING, False)
                    proj_chunk(xT_k, kpT, 512, 512, b2s_sb, SCALING, False)
                    proj_chunk(xT_k, kpT, 1024, 32, b2s_sb, SCALING, False)
                    for i in range(9):
                        vproj_chunk(i)
                    if stage < 3.5:
                        debug_dump(qpT[:, 0, 0:256], 256)
                    else:
                        nc.sync.dma_start(
                            out=out_d.ap()[0:128][:, 0:32],
                            in_=vpa[0][:, 0, 0:32],
                        )
                    continue

                # ---- interleaved proj + scores + exp + mask + out ----------
                # Lead-in projects the first 512 tokens of q/k; remaining
                # proj chunks and all vproj chunks are issued inside the
                # score loop so PE stays ahead of Act's exp chain and Act's
                # exp(c+1) never waits behind out-matmuls (out_tile trails
                # by 2 chunks).
                aw = [
                    aw_pool.tile(
                        [128, 4, 2, 160], BF16, tag=f"aw{c}", name=f"aw{c}"
                    )
                    for c in range(NKTILE)
                ]
                obuf = ofin_pool.tile([128, 8, 256], BF16, tag="obuf")

                def out_tile(t):
                    po = ps_o.tile(
                        [128, H, 33], F32, tag="po", name="po",
                        padded_shape=[128, H, 64], bufs=2,
                    )
                    for h in range(H):
                        hr, ht = h % 4, h // 4
                        nc.tensor.matmul(
                            po[:, h, 0:33],
                            aw[t][:, hr, ht, 32:160],
                            vpa[t][:, h, :],
                            start=True,
                            stop=False,
                            tile_position=(0, 0),
                            skip_group_check=True,
                        )
                        nc.tensor.matmul(
                            po[96:128, h, 0:33],
                            aw[t + 1][0:32, hr, ht, 0:32],
                            vpa[t + 1][0:32, h, :],
                            start=False,
                            stop=True,
                            tile_position=(0, 96),
                            skip_group_check=True,
                        )
                    rec = ofin_pool.tile([128, H], F32, tag="rec")
                    nc.vector.reciprocal(rec, po[:, :, 32])
                    rec_b = bass.AP(
                        tensor=rec.tensor,
                        offset=rec.offset,
                        ap=[rec.ap[0], [rec.ap[1][0], H], [0, 32]],
                    )
                    nc.vector.tensor_mul(
                        obuf[:, t, :].rearrange("p (h x) -> p h x", h=H),
                        po[:, :, 0:32],
                        rec_b,
                    )
                    if t in (1, 3, 5):
                        nc.sync.dma_start(
                            out=out_d.ap().rearrange("(c p) e -> p c e", p=128)[
                                :, t - 1 : t + 1, :
                            ],
                            in_=obuf[:, t - 1 : t + 1, :],
                        )

                proj_chunk(xT_q, qpT, 0, 512, b2_sb, 1.0, True)
                proj_chunk(xT_k, kpT, 0, 512, b2s_sb, SCALING, True)

                for c in range(NKTILE):
                    if c == 4 and _rep + 1 < reps:
                        nxt = xT_pool.tile([128, 2, XN], BF16, tag="xT", name="nxt")
                        nc.sync.dma_start_transpose(nxt, xin_d.ap())
                        xT_next.append(nxt)
                    if c == 1:
                        proj_chunk(xT_q, qpT, 512, 512, b2_sb, 1.0, True)
                    elif c == 2:
                        proj_chunk(xT_k, kpT, 512, 512, b2s_sb, SCALING, True)
                    elif c == 3:
                        proj_chunk(xT_k, kpT, 1024, 32, b2s_sb, SCALING, False)
                    vproj_chunk(c)

                    ps = ps_s.tile(
                        [128, 4, 2, 160], F32, tag="pss",
                        padded_shape=[128, 4, 2, 256],
                    )
                    if c == 0:
                        i0, i1 = 32, 160  # window cols < 32 are q < 0
                        q0, q1 = 0, 128
                    elif c == NKTILE - 1:
                        i0, i1 = 0, 32  # only q blocks up to 1024 exist
                        q0, q1 = 992, 1024
                    else:
                        i0, i1 = 0, 160
                        q0, q1 = 128 * c - 32, 128 * c + 128
                    kw = 128 if c < NKTILE - 1 else 32
                    for h in range(H):
                        hr, ht = h % 4, h // 4
                        nc.tensor.matmul(
                            ps[0:kw, hr, ht, i0:i1],
                            kpT[
                                32 * hr : 32 * hr + 32, ht,
                                128 * c : 128 * c + kw,
                            ],
                            qpT[32 * hr : 32 * hr + 32, ht, q0:q1],
                            start=True,
                            stop=True,
                            tile_position=(32 * hr, 0),
                        )
                    ex = aw_pool.tile(
                        [128, 4, 2, 160], BF16, tag="ex", bufs=3
                    )
                    pp = 128 if c < NKTILE - 1 else 32
                    nc.scalar.activation(
                        ex[:pp, :, :, i0:i1],
                        ps[:pp, :, :, i0:i1],
                        mybir.ActivationFunctionType.Exp,
                    )
                    # masks all on Pool (SBUF-only engine; Act is exp-bound,
                    # DVE drain-bound).  First-32-col slice first so the
                    # trailing out_tile's halo matmuls unblock early.
                    if i0 == 0:
                        nc.gpsimd.tensor_mul(
                            aw[c][:pp, :, :, 0:32],
                            ex[:pp, :, :, 0:32],
                            bass.AP(
                                tensor=mask_sb.tensor,
                                offset=mask_sb.offset,
                                ap=[[mask_sb.ap[0][0], pp], [0, 4], [0, 2], [1, 32]],
                            ),
                        )
                        if stage >= 4 and c >= 2:
                            out_tile(c - 2)
                        if i1 > 32:
                            nc.gpsimd.tensor_mul(
                                aw[c][:pp, :, :, 32:160],
                                ex[:pp, :, :, 32:160],
                                mask_bcast(32, 128),
                            )
                    else:
                        nc.gpsimd.tensor_mul(
                            aw[c][:pp, :, :, i0:i1],
                            ex[:pp, :, :, i0:i1],
                            mask_bcast(i0, i1 - i0),
                        )
                        if stage >= 4 and c >= 2:
                            out_tile(c - 2)

                if stage >= 4:
                    out_tile(NKTILE - 2)
                    nc.sync.dma_start(
                        out=out_d.ap().rearrange("(c p) e -> p c e", p=128)[
                            :, 6:8, :
                        ],
                        in_=obuf[:, 6:8, :],
                    )
                else:
                    nc.sync.dma_start(
                        out=out_d.ap()[0:128][:, 0:128],
                        in_=aw[0][:, 0, 0, 32:160],
                    )

    nc.compile()
    return nc


_NC_CACHE = {}


def _get_nc(bias_zero):
    if bias_zero not in _NC_CACHE:
        _NC_CACHE[bias_zero] = build_program(bias_zero=bias_zero)
    return _NC_CACHE[bias_zero]


def make_in_maps(query, key, value, W, b):
    query = np.asarray(query, np.float32)
    key = np.asarray(key, np.float32)
    value = np.asarray(value, np.float32)
    W = np.asarray(W, np.float32)
    b = np.asarray(b, np.float32)

    wT = np.ascontiguousarray(W.T).reshape(2, 128, 256).astype(NPBF16)
    b2 = b.reshape(2, 128).astype(np.float32)
    b2s = (SCALING * b2).astype(np.float32)
    brow = b.reshape(1, 256).astype(NPBF16)
    mask = _host_mask()
    ones_row = np.ones((1, 128), NPBF16)

    qf = query.reshape(B * N, E).astype(NPBF16)
    kf = key.reshape(B * N, E).astype(NPBF16)
    vf = value.reshape(B * N, E).astype(NPBF16)
    shards_per_b = NCORES // B
    in_maps = []
    for c in range(NCORES):
        s0 = c * T
        if c % shards_per_b == 0:
            halo_k = np.zeros((32, E), NPBF16)
            halo_v = np.zeros((32, E), NPBF16)
        else:
            halo_k = kf[s0 - 32 : s0]
            halo_v = vf[s0 - 32 : s0]
        xin = np.concatenate(
            [qf[s0 : s0 + T], halo_k, kf[s0 : s0 + T], halo_v, vf[s0 : s0 + T]]
        )  # [3136, 256]
        in_maps.append(
            {
                "xin": xin,
                "wT": wT,
                "b2": b2,
                "b2s": b2s,
                "brow": brow,
                "ones_row": ones_row,
                "mask": mask,
            }
        )
    return in_maps


def kernel(query, key, value, W, b):
    nc = _get_nc(not np.any(np.asarray(b)))
    in_maps = make_in_maps(query, key, value, W, b)
    res = run_bass_kernel_spmd(nc, in_maps, list(range(NCORES)))
    out = np.concatenate(
        [np.asarray(res.results[c]["out"]).astype(np.float32) for c in range(NCORES)],
        axis=0,
    )
    return out.reshape(B, N, E)
